# revision 13
# baseline (speedup 1.0000x reference)
"""Trainium2 Bass kernel for nn_FeatureContraction.

Computes out[b,c,w,x,v] = sum_i x[b,c,w,x,v,i] * node_attributes[b,c,i]
with B=C=128, X=3, Y=16 (wxv = 3*16*16 = 768, i = 16).

Strategy (8 NeuronCores, data-parallel over b):
  - each core owns 16 b-slices; x-shard is [16, 128, 768, 16] f32 (96 MiB)
  - SBUF layout: partitions = c (128), free = contiguous (wxv, i)
    -> DMA reads 48 KiB contiguous per partition (full HBM rate).
    The load casts f32 -> bf16 in the DMA datapath (SWDGE cast).
  - multiply: tmp[c, w, i] = x[c, w, i] * na[c, i] with a step-0
    broadcast AP on na (DVE 2x mode, contiguous streams).
  - reduce over i, split by w to balance engines:
      w < RED_SPLIT: DVE grouped tensor_reduce (innermost axis)
      w >= RED_SPLIT: 16 identity-weight PE matmuls accumulating the
      strided i-slices into PSUM, then ACT copies PSUM->SBUF.
  - the last b-slice is loaded in two halves so the pipeline tail is
    short (the DVE half finishes last).
This keeps the kernel at the HBM roofline (~100 MiB/core of traffic).
"""

import sys

for _p in ("/opt/trn_rl_repo",):
    if _p not in sys.path:
        sys.path.append(_p)

import numpy as np

import concourse.bass as bass
import concourse.mybir as mybir
import concourse.tile as tile
from concourse import bacc
from concourse.bass_utils import run_bass_kernel_spmd

# Problem dims (hardcoded per spec)
B, C, X, Y = 128, 128, 3, 16
WXV = X * Y * Y          # 768
I = Y                    # 16 (contraction axis)
N_CORES = 8
B_LOC = B // N_CORES     # 16 b-slices per core

RED_SPLIT = 336          # DVE reduces w < RED_SPLIT, PE reduces the rest

F32 = mybir.dt.float32
BF16 = mybir.dt.bfloat16

_COMPILED = None


def _build():
    nc = bacc.Bacc("TRN2", target_bir_lowering=False, debug=False,
                   num_devices=N_CORES)

    x_d = nc.dram_tensor("x", [B_LOC, C, WXV, I], F32, kind="ExternalInput")
    na_d = nc.dram_tensor("naT", [C, B_LOC, I], F32, kind="ExternalInput")
    eye_d = nc.dram_tensor("eye", [C, C], F32, kind="ExternalInput")
    out_d = nc.dram_tensor("out", [B_LOC, C, WXV], F32, kind="ExternalOutput")

    WA = RED_SPLIT
    WB = WXV - RED_SPLIT

    with tile.TileContext(nc) as tc:
        with (
            tc.tile_pool(name="const", bufs=1) as constp,
            tc.tile_pool(name="xp", bufs=3) as xp,
            tc.tile_pool(name="tmpp", bufs=3) as tmpp,
            tc.tile_pool(name="outp", bufs=3) as outp,
            tc.tile_pool(name="psp", bufs=4, space="PSUM") as psp,
        ):
            eye = constp.tile([C, C], BF16)
            na_sb = constp.tile([C, B_LOC, I], BF16)
            eye_f = constp.tile([C, C], F32)
            na_f = constp.tile([C, B_LOC, I], F32)

            def compute(b, xt_b, xt_a, oa_ap, ob_ap):
                nab = na_sb[:, b, :][:, None, :]
                # B half: mult then 16 PE identity matmuls (psum accumulate)
                tb = tmpp.tile([C, WB, I], BF16, tag="tmpb")
                nc.vector.tensor_mul(tb[:], xt_b,
                                     nab.broadcast_to([C, WB, I]))
                ps = psp.tile([C, WB], F32, tag="ps")
                for i in range(I):
                    nc.tensor.matmul(ps[:], eye[:], tb[:, :, i],
                                     start=(i == 0), stop=(i == I - 1))
                # A half: mult then DVE grouped reduce
                ta = tmpp.tile([C, WA, I], BF16, tag="tmpa")
                nc.vector.tensor_mul(ta[:], xt_a,
                                     nab.broadcast_to([C, WA, I]))
                nc.scalar.copy(ob_ap, ps[:])
                nc.vector.tensor_reduce(oa_ap, ta[:], mybir.AxisListType.X,
                                        mybir.AluOpType.add)

            for b in range(B_LOC - 1):
                xt = xp.tile([C, WXV, I], BF16, tag="x")
                nc.gpsimd.dma_start(xt[:], x_d[b])  # f32 -> bf16 cast
                if b == 0:
                    # constants via the idle HWDGE ring (keeps Q7 on x loads),
                    # converted to bf16 on DVE
                    nc.sync.dma_start(eye_f[:], eye_d[:])
                    nc.sync.dma_start(na_f[:], na_d[:])
                    nc.vector.tensor_copy(eye[:], eye_f[:])
                    nc.vector.tensor_copy(na_sb[:], na_f[:])
                ot = outp.tile([C, WXV], F32, tag="out")
                compute(b, xt[:, RED_SPLIT:, :], xt[:, :RED_SPLIT, :],
                        ot[:, :RED_SPLIT], ot[:, RED_SPLIT:])
                nc.scalar.dma_start(out_d[b], ot[:])

            # last b-slice: four quarter items so the pipeline tail is one
            # quarter's compute; alternate PE / DVE quarters
            b = B_LOC - 1
            nab = na_sb[:, b, :][:, None, :]
            ot = outp.tile([C, WXV], F32, tag="out")
            quarters = [
                ("pe", 384, 576), ("dve", 0, 192),
                ("pe", 576, 768), ("dve", 192, 384),
            ]
            for kind, w0, w1 in quarters:
                wq = w1 - w0
                xq = xp.tile([C, wq, I], BF16, tag="x")
                nc.gpsimd.dma_start(xq[:], x_d[b, :, w0:w1, :])
                tq = tmpp.tile([C, wq, I], BF16, tag="tmpb")
                nc.vector.tensor_mul(tq[:], xq[:],
                                     nab.broadcast_to([C, wq, I]))
                if kind == "pe":
                    ps = psp.tile([C, wq], F32, tag="ps")
                    for i in range(I):
                        nc.tensor.matmul(ps[:], eye[:], tq[:, :, i],
                                         start=(i == 0), stop=(i == I - 1))
                    nc.scalar.copy(ot[:, w0:w1], ps[:])
                else:
                    nc.vector.tensor_reduce(ot[:, w0:w1], tq[:],
                                            mybir.AxisListType.X,
                                            mybir.AluOpType.add)
                nc.scalar.dma_start(out_d[b, :, w0:w1], ot[:, w0:w1])

    nc.compile()
    return nc


def _get_compiled():
    global _COMPILED
    if _COMPILED is None:
        _COMPILED = _build()
    return _COMPILED


def _make_in_maps(inputs: dict):
    x = np.ascontiguousarray(np.asarray(inputs["x"], dtype=np.float32))
    na = np.asarray(inputs["node_attributes"], dtype=np.float32)

    x_sh = x.reshape(B, C, WXV, I)
    naT = np.ascontiguousarray(na.transpose(1, 0, 2))  # [C, B, I]
    eye = np.eye(C, dtype=np.float32)

    in_maps = []
    for k in range(N_CORES):
        b0 = k * B_LOC
        in_maps.append(
            {
                "x": x_sh[b0 : b0 + B_LOC],
                "naT": np.ascontiguousarray(naT[:, b0 : b0 + B_LOC, :]),
                "eye": eye,
            }
        )
    return in_maps


def _gather(results) -> np.ndarray:
    out = np.concatenate([r["out"] for r in results], axis=0)
    return out.reshape(B, C, X, Y, Y)


def _run(inputs: dict, trace: bool = False, trace_cores=None):
    in_maps = _make_in_maps(inputs)
    nc = _get_compiled()
    res = run_bass_kernel_spmd(
        nc,
        in_maps,
        core_ids=list(range(N_CORES)),
        trace=trace,
        trace_cores=trace_cores,
    )
    return _gather(res.results), res


def kernel(**inputs) -> np.ndarray:
    out, _ = _run(inputs, trace=False)
    return out


# revision 14
# speedup vs baseline: 1.0450x; 1.0450x over previous
"""Trainium2 Bass kernel for nn_FeatureContraction.

Computes out[b,c,w,x,v] = sum_i x[b,c,w,x,v,i] * node_attributes[b,c,i]
with B=C=128, X=3, Y=16 (wxv = 3*16*16 = 768, i = 16).

Strategy (8 NeuronCores, data-parallel over b):
  - each core owns 16 b-slices; x-shard is [16, 128, 768, 16] f32 (96 MiB)
  - SBUF layout: partitions = c (128), free = contiguous (wxv, i)
    -> DMA reads 48 KiB contiguous per partition (full HBM rate).
    The load casts f32 -> bf16 in the DMA datapath (SWDGE cast).
  - multiply: tmp[c, w, i] = x[c, w, i] * na[c, i] with a step-0
    broadcast AP on na (DVE 2x mode, contiguous streams).
  - reduce over i, split by w to balance engines:
      w < RED_SPLIT: DVE grouped tensor_reduce (innermost axis)
      w >= RED_SPLIT: 16 identity-weight PE matmuls accumulating the
      strided i-slices into PSUM, then ACT copies PSUM->SBUF.
  - the last b-slice is loaded in two halves so the pipeline tail is
    short (the DVE half finishes last).
This keeps the kernel at the HBM roofline (~100 MiB/core of traffic).
"""

import sys

for _p in ("/opt/trn_rl_repo",):
    if _p not in sys.path:
        sys.path.append(_p)

import numpy as np

import concourse.bass as bass
import concourse.mybir as mybir
import concourse.tile as tile
from concourse import bacc
from concourse.bass_utils import run_bass_kernel_spmd

# Problem dims (hardcoded per spec)
B, C, X, Y = 128, 128, 3, 16
WXV = X * Y * Y          # 768
I = Y                    # 16 (contraction axis)
N_CORES = 8
B_LOC = B // N_CORES     # 16 b-slices per core

RED_SPLIT = 336          # DVE reduces w < RED_SPLIT, PE reduces the rest

F32 = mybir.dt.float32
BF16 = mybir.dt.bfloat16

_COMPILED = None


def _build():
    nc = bacc.Bacc("TRN2", target_bir_lowering=False, debug=False,
                   num_devices=N_CORES)

    x_d = nc.dram_tensor("x", [B_LOC, C, WXV, I], F32, kind="ExternalInput")
    na_d = nc.dram_tensor("naT", [C, B_LOC, I], F32, kind="ExternalInput")
    eye_d = nc.dram_tensor("eye", [C, C], F32, kind="ExternalInput")
    out_d = nc.dram_tensor("out", [B_LOC, C, WXV], F32, kind="ExternalOutput")

    WA = RED_SPLIT
    WB = WXV - RED_SPLIT

    with tile.TileContext(nc) as tc:
        with (
            tc.tile_pool(name="const", bufs=1) as constp,
            tc.tile_pool(name="xp", bufs=3) as xp,
            tc.tile_pool(name="tmpp", bufs=3) as tmpp,
            tc.tile_pool(name="outp", bufs=3) as outp,
            tc.tile_pool(name="psp", bufs=4, space="PSUM") as psp,
        ):
            eye = constp.tile([C, C], BF16)
            na_sb = constp.tile([C, B_LOC, I], BF16)
            eye_f = constp.tile([C, C], F32)
            na_f = constp.tile([C, B_LOC, I], F32)

            def compute(b, xt_b, xt_a, oa_ap, ob_ap):
                nab = na_sb[:, b, :][:, None, :]
                # B half: mult then 16 PE identity matmuls (psum accumulate)
                tb = tmpp.tile([C, WB, I], BF16, tag="tmpb")
                nc.vector.tensor_mul(tb[:], xt_b,
                                     nab.broadcast_to([C, WB, I]))
                ps = psp.tile([C, WB], F32, tag="ps")
                for i in range(I):
                    nc.tensor.matmul(ps[:], eye[:], tb[:, :, i],
                                     start=(i == 0), stop=(i == I - 1))
                # A half: mult then DVE grouped reduce
                ta = tmpp.tile([C, WA, I], BF16, tag="tmpa")
                nc.vector.tensor_mul(ta[:], xt_a,
                                     nab.broadcast_to([C, WA, I]))
                nc.scalar.copy(ob_ap, ps[:])
                nc.vector.tensor_reduce(oa_ap, ta[:], mybir.AxisListType.X,
                                        mybir.AluOpType.add)

            for b in range(B_LOC - 1):
                xt = xp.tile([C, WXV, I], BF16, tag="x")
                nc.gpsimd.dma_start(xt[:], x_d[b])  # f32 -> bf16 cast
                if b == 0:
                    # constants via the idle HWDGE ring (keeps Q7 on x loads),
                    # converted to bf16 on DVE
                    nc.sync.dma_start(eye_f[:], eye_d[:])
                    nc.sync.dma_start(na_f[:], na_d[:])
                    nc.vector.tensor_copy(eye[:], eye_f[:])
                    nc.vector.tensor_copy(na_sb[:], na_f[:])
                ot = outp.tile([C, WXV], F32, tag="out")
                compute(b, xt[:, RED_SPLIT:, :], xt[:, :RED_SPLIT, :],
                        ot[:, :RED_SPLIT], ot[:, RED_SPLIT:])
                nc.scalar.dma_start(out_d[b], ot[:])

            # last b-slice: two half loads for a short pipeline tail
            b = B_LOC - 1
            xb = xp.tile([C, WB, I], BF16, tag="x")
            nc.gpsimd.dma_start(xb[:], x_d[b, :, RED_SPLIT:, :])
            xa = xp.tile([C, WA, I], BF16, tag="x")
            nc.gpsimd.dma_start(xa[:], x_d[b, :, :RED_SPLIT, :])
            ot = outp.tile([C, WXV], F32, tag="out")
            compute(b, xb[:], xa[:], ot[:, :RED_SPLIT], ot[:, RED_SPLIT:])
            nc.scalar.dma_start(out_d[b, :, RED_SPLIT:], ot[:, RED_SPLIT:])
            nc.scalar.dma_start(out_d[b, :, :RED_SPLIT], ot[:, :RED_SPLIT])

    nc.compile()
    return nc


def _get_compiled():
    global _COMPILED
    if _COMPILED is None:
        _COMPILED = _build()
    return _COMPILED


def _make_in_maps(inputs: dict):
    x = np.ascontiguousarray(np.asarray(inputs["x"], dtype=np.float32))
    na = np.asarray(inputs["node_attributes"], dtype=np.float32)

    x_sh = x.reshape(B, C, WXV, I)
    naT = np.ascontiguousarray(na.transpose(1, 0, 2))  # [C, B, I]
    eye = np.eye(C, dtype=np.float32)

    in_maps = []
    for k in range(N_CORES):
        b0 = k * B_LOC
        in_maps.append(
            {
                "x": x_sh[b0 : b0 + B_LOC],
                "naT": np.ascontiguousarray(naT[:, b0 : b0 + B_LOC, :]),
                "eye": eye,
            }
        )
    return in_maps


def _gather(results) -> np.ndarray:
    out = np.concatenate([r["out"] for r in results], axis=0)
    return out.reshape(B, C, X, Y, Y)


def _run(inputs: dict, trace: bool = False, trace_cores=None):
    in_maps = _make_in_maps(inputs)
    nc = _get_compiled()
    res = run_bass_kernel_spmd(
        nc,
        in_maps,
        core_ids=list(range(N_CORES)),
        trace=trace,
        trace_cores=trace_cores,
    )
    return _gather(res.results), res


def kernel(**inputs) -> np.ndarray:
    out, _ = _run(inputs, trace=False)
    return out
